# revision 23
# baseline (speedup 1.0000x reference)
"""AttentionPool Trainium2 kernel (v3).

Computes, for x (B,T,m), W1 (m,m), W2 (m,m), vm (1,m):
    h      = tanh(x @ W1 + vm @ W2)          (B,T,m)
    scores = h @ vm[0]                       (B,T,1)
    w      = softmax(scores, axis=T)
    out    = sum(x * w, axis=T, keepdims)    (B,1,m)

Sharding: data-parallel over B across 8 NeuronCores (2 examples per core);
W1/W2/vm replicated.  Softmax needs no max-subtraction: |scores| <= ||vm||_1
(~13 at this problem scale), safely inside fp32/bf16 exp range.

v3 layout (x cast to bf16 on host, plus a host-transposed copy xT; rel-err
budget 2e-2, measured ~2.5e-3):
  - 1024-row t-chunks; DMA granule = 2 chunks so every descriptor is 4 KiB.
  - xin [p,cc,r,m]: t-partitioned (t = c*1024 + p*8 + r), for pooling.
  - xts [p,mh,tau]: m-partitioned from xT, for the score path.
  - h^T = W1^T @ x^T on PE (bf16), split in 512-column halves to fit two
    double-buffered PSUM banks; tanh+bias on ACT per half.
  - scores: per 128-t block, stationary strided hsb slice (tau = q*8 + r)
    so score partitions line up with xin's t layout; rhs = vm column.
  - e = exp(s) -> e_all (bf16).
  - pooling on PE: acc_ps[m-half] += xin^T @ e_col, 1-column accumulating
    matmuls with x stationary (PSUM memset once; start=True would zero the
    whole bank row and wipe the sibling m-half chain).
  - software pipelining: scores/exp lag h by one chunk, pooling by two.
"""

import numpy as np
import ml_dtypes

import concourse.bass as bass
import concourse.tile as tile
from concourse import bacc, mybir
from concourse.bass_utils import run_bass_kernel_spmd

FP32 = mybir.dt.float32
BF16 = mybir.dt.bfloat16
AF = mybir.ActivationFunctionType

N_CORES = 8
B = 16
B_PER_CORE = B // N_CORES  # 2
T = 8192
M = 256
P = 128
CHUNK = 1024         # t rows per chunk
NT = CHUNK // P      # 8 t-rows per partition per chunk
NCHUNK = T // CHUNK  # 8 chunks per example
NE = NCHUNK * NT     # e columns per example (64)
GC = 2               # chunks per DMA granule
GRAN = GC * CHUNK    # 2048 t rows per granule
NGRAN = T // GRAN    # 4 granules per example
HALF = CHUNK // 2    # h-matmul column split (PSUM bank budget)


def _build_program() -> bass.Bass:
    nc = bacc.Bacc("TRN2", target_bir_lowering=False, debug=False)

    x = nc.dram_tensor("x", [B_PER_CORE, T, M], BF16, kind="ExternalInput")
    xT = nc.dram_tensor("xT", [B_PER_CORE, M, T], BF16, kind="ExternalInput")
    W1 = nc.dram_tensor("W1", [M, M], FP32, kind="ExternalInput")
    W2 = nc.dram_tensor("W2", [M, M], FP32, kind="ExternalInput")
    vm = nc.dram_tensor("vm", [1, M], FP32, kind="ExternalInput")
    out = nc.dram_tensor("out", [B_PER_CORE, M], FP32, kind="ExternalOutput")

    with tile.TileContext(nc) as tc:
        with (
            tc.tile_pool(name="setup", bufs=1) as setup,
            tc.tile_pool(name="xin", bufs=4) as xin_pool,
            tc.tile_pool(name="xts", bufs=4) as xts_pool,
            tc.tile_pool(name="hps", bufs=2, space="PSUM") as hps_pool,
            tc.tile_pool(name="hsb", bufs=2) as hsb_pool,
            tc.tile_pool(name="sps", bufs=2, space="PSUM") as sps_pool,
            tc.tile_pool(name="acc", bufs=1, space="PSUM") as acc_pool,
            tc.tile_pool(name="eee", bufs=1) as e_pool,
            tc.tile_pool(name="fin", bufs=2) as fin_pool,
        ):
            # ---------------- setup ----------------
            # W1 blocks: w1b[p, i, n] = W1[i*128+p, n], cast to bf16
            w1f = setup.tile([P, 2, M], FP32)
            nc.sync.dma_start(out=w1f, in_=W1.rearrange("(a p) n -> p a n", p=P))
            w1b = setup.tile([P, 2, M], BF16)
            nc.vector.tensor_copy(w1b, w1f)

            # W2 blocks (f32, setup only)
            w2f = setup.tile([P, 2, M], FP32)
            nc.sync.dma_start(out=w2f, in_=W2.rearrange("(a p) n -> p a n", p=P))

            # vm transposed: vmt[p, i] = vm[0, i*128+p]
            vmt_f = setup.tile([P, 2], FP32)
            nc.sync.dma_start(out=vmt_f, in_=vm[0].rearrange("(a p) -> p a", p=P))
            vmt_b = setup.tile([P, 2], BF16)
            nc.vector.tensor_copy(vmt_b, vmt_f)

            # c = vm @ W2, computed directly transposed: c_sb[p, nh] = c[nh*128+p]
            c_ps = sps_pool.tile([P, 2], FP32, tag="sps")
            for nh in range(2):
                for mh in range(2):
                    nc.tensor.matmul(
                        c_ps[:, nh : nh + 1],
                        lhsT=w2f[:, mh, nh * P : (nh + 1) * P],
                        rhs=vmt_f[:, mh : mh + 1],
                        start=(mh == 0),
                        stop=(mh == 1),
                    )
            c_sb = setup.tile([P, 2], FP32)
            nc.vector.tensor_copy(c_sb, c_ps)

            ones_col = setup.tile([P, 1], FP32)
            nc.vector.memset(ones_col, 1.0)
            ones_row = setup.tile([1, P], FP32)
            nc.vector.memset(ones_row, 1.0)

            # ---------------- main loop ----------------
            # One flat pipeline over B_PER_CORE * NCHUNK chunks with
            # granule-ahead DMA prefetch (also across the example boundary).
            NCT = B_PER_CORE * NCHUNK   # 16 global chunks
            NGT = B_PER_CORE * NGRAN    # 8 global granules

            xin_t = [None] * NCT
            xts_t = [None] * NCT
            hsb_t = [None] * NCT
            e_t = [None] * B_PER_CORE
            acc_t = [None] * B_PER_CORE

            def emit_dma(g, split=False):
                b, gb = divmod(g, NGRAN)
                # granule of 2 chunks; every descriptor 4 KiB contiguous.
                # xts first: the h-matmuls only need xts, xin is not read
                # until the pooling two chunks later.
                # split=True issues per-chunk transfers (first granule only)
                # so the pipeline can start on chunk 0 without waiting for
                # the whole granule.
                # xts[p, mh, tau] = x[b, gb*2048+tau, mh*128+p]
                xts = xts_pool.tile([P, 2, GRAN], BF16)
                if split:
                    for cc in range(GC):
                        xc = slice(cc * CHUNK, (cc + 1) * CHUNK)
                        xts_c = xts_pool.tile([P, 2, CHUNK], BF16, tag=f"xts0{cc}")
                        nc.sync.dma_start(
                            out=xts_c,
                            in_=xT[b, :, gb * GRAN + cc * CHUNK :
                                   gb * GRAN + (cc + 1) * CHUNK].rearrange(
                                "(a p) t -> p a t", p=P
                            ),
                        )
                        xts_t[g * GC + cc] = xts_c
                else:
                    nc.sync.dma_start(
                        out=xts,
                        in_=xT[b, :, gb * GRAN : (gb + 1) * GRAN].rearrange(
                            "(a p) t -> p a t", p=P
                        ),
                    )
                    for cc in range(GC):
                        xts_t[g * GC + cc] = xts[:, :, cc * CHUNK : (cc + 1) * CHUNK]
                # xin[p, cc, r, m] = x[b, gb*2048 + cc*1024 + p*8 + r, m]
                xin = xin_pool.tile([P, GC, NT, M], BF16)
                nc.sync.dma_start(
                    out=xin,
                    in_=x[b, gb * GRAN : (gb + 1) * GRAN, :].rearrange(
                        "(cc p r) m -> p cc r m", p=P, r=NT
                    ),
                )
                for cc in range(GC):
                    xin_t[g * GC + cc] = xin[:, cc]

            def emit_h(ct):
                xts = xts_t[ct]
                hsb = hsb_pool.tile([P, 2, P, NT], BF16)
                for hf in range(2):
                    # h^T = W1^T @ x^T, 512-column half, acc over m-halves
                    hps = hps_pool.tile([P, 2, HALF], FP32)
                    for nh in range(2):
                        for mh in range(2):
                            nc.tensor.matmul(
                                hps[:, nh, :],
                                lhsT=w1b[:, mh, nh * P : (nh + 1) * P],
                                rhs=xts[:, mh, hf * HALF : (hf + 1) * HALF],
                                start=(mh == 0),
                                stop=(mh == 1),
                            )
                    # tanh with per-partition bias c into hsb[p, nh, q, r]
                    # over tau = q*8 + r (this half: q in [64*hf, 64*hf+64))
                    q0 = hf * (P // 2)
                    for nh in range(2):
                        nc.scalar.activation(
                            hsb[:, nh, q0 : q0 + P // 2, :],
                            hps[:, nh],
                            AF.Tanh,
                            bias=c_sb[:, nh : nh + 1],
                        )
                hsb_t[ct] = hsb

            def emit_scores(ct):
                # s[q, r] for t = c*1024 + q*8 + r: stationary strided hsb
                # slice [128 tau = q*8+r], moving vm column
                b, c = divmod(ct, NCHUNK)
                e_all = e_t[b]
                sps = sps_pool.tile([P, NT], FP32, tag="sps")
                hsb = hsb_t[ct]
                for r in range(NT):
                    for nh in range(2):
                        nc.tensor.matmul(
                            sps[:, r : r + 1],
                            lhsT=hsb[:, nh, :, r],
                            rhs=vmt_b[:, nh : nh + 1],
                            start=(nh == 0),
                            stop=(nh == 1),
                        )
                nc.scalar.activation(
                    e_all[:, c * NT : (c + 1) * NT],
                    sps,
                    AF.Exp,
                )
                hsb_t[ct] = None

            def emit_pool(ct):
                # acc_ps[q, mh] += sum_p x[t(p,r), mh*128+q] * e[t(p,r)]
                b, c = divmod(ct, NCHUNK)
                e_all = e_t[b]
                acc_ps = acc_t[b]
                xin = xin_t[ct]
                for r in range(NT):
                    for mh in range(2):
                        nc.tensor.matmul(
                            acc_ps[:, mh : mh + 1],
                            lhsT=xin[:, r, mh * P : (mh + 1) * P],
                            rhs=e_all[:, c * NT + r : c * NT + r + 1],
                            start=False,
                            stop=(c == NCHUNK - 1 and r == NT - 1),
                            skip_group_check=True,
                        )
                xin_t[ct] = None

            def emit_finalize(b):
                e_all = e_t[b]
                acc_ps = acc_t[b]
                # Z = sum(e_all): free-dim reduce on DVE, partition reduce on PE
                z_red = fin_pool.tile([P, 1], FP32)
                nc.vector.reduce_sum(z_red, e_all, axis=mybir.AxisListType.X)
                z_ps = sps_pool.tile([1, 1], FP32, tag="sps")
                nc.tensor.matmul(z_ps, lhsT=z_red, rhs=ones_col, start=True, stop=True)
                z_sb = fin_pool.tile([1, 1], FP32)
                nc.vector.tensor_copy(z_sb, z_ps)
                # broadcast Z to all partitions, then reciprocal
                zb_ps = sps_pool.tile([P, 1], FP32, tag="sps")
                nc.tensor.matmul(zb_ps, lhsT=ones_row, rhs=z_sb, start=True, stop=True)
                rz = fin_pool.tile([P, 1], FP32)
                nc.vector.reciprocal(rz, zb_ps)
                # scale pooled sums by 1/Z; acc_ps is already m-partitioned
                outsb = fin_pool.tile([P, 2], FP32)
                nc.vector.tensor_scalar_mul(outsb, acc_ps, rz)
                nc.sync.dma_start(
                    out=out[b].rearrange("(a p) -> p a", p=P), in_=outsb
                )

            for b in range(B_PER_CORE):
                e_t[b] = e_pool.tile([P, NE], BF16, name=f"e_all{b}")
                # start=True zeroes the whole PSUM bank row, so the two
                # m-half accumulation chains sharing this tile would wipe
                # each other; memset once and accumulate-only instead.
                acc_t[b] = acc_pool.tile([P, 2], FP32, name=f"acc{b}")
                nc.vector.memset(acc_t[b], 0.0)

            emit_dma(0, split=True)
            for g in range(1, 3):
                emit_dma(g)
            for ct in range(NCT):
                if ct % GC == 0 and ct // GC + 3 < NGT:
                    emit_dma(ct // GC + 3)
                emit_h(ct)
                if ct >= 1:
                    emit_scores(ct - 1)
                if ct >= 2:
                    emit_pool(ct - 2)
                if ct == NCHUNK + 1:
                    emit_finalize(0)
            emit_scores(NCT - 1)
            emit_pool(NCT - 2)
            emit_pool(NCT - 1)
            emit_finalize(1)

    return nc


_PROGRAM_CACHE: list = []


def _get_program() -> bass.Bass:
    if not _PROGRAM_CACHE:
        nc = _build_program()
        nc.finalize()
        _PROGRAM_CACHE.append(nc)
    return _PROGRAM_CACHE[0]


def _make_in_maps(x, W1, W2, vm):
    xb = np.ascontiguousarray(x).astype(ml_dtypes.bfloat16)
    xbT = np.ascontiguousarray(xb.transpose(0, 2, 1))
    W1 = np.ascontiguousarray(W1, dtype=np.float32)
    W2 = np.ascontiguousarray(W2, dtype=np.float32)
    vm = np.ascontiguousarray(vm, dtype=np.float32)
    return [
        {
            "x": xb[i * B_PER_CORE : (i + 1) * B_PER_CORE],
            "xT": xbT[i * B_PER_CORE : (i + 1) * B_PER_CORE],
            "W1": W1,
            "W2": W2,
            "vm": vm,
        }
        for i in range(N_CORES)
    ]


def kernel(x, W1, W2, vm):
    nc = _get_program()
    in_maps = _make_in_maps(x, W1, W2, vm)
    res = run_bass_kernel_spmd(nc, in_maps, list(range(N_CORES)))
    out = np.concatenate([res.results[i]["out"] for i in range(N_CORES)], axis=0)
    return out.reshape(B, 1, M).astype(np.float32)


# revision 24
# speedup vs baseline: 1.0391x; 1.0391x over previous
"""AttentionPool Trainium2 kernel (v3).

Computes, for x (B,T,m), W1 (m,m), W2 (m,m), vm (1,m):
    h      = tanh(x @ W1 + vm @ W2)          (B,T,m)
    scores = h @ vm[0]                       (B,T,1)
    w      = softmax(scores, axis=T)
    out    = sum(x * w, axis=T, keepdims)    (B,1,m)

Sharding: data-parallel over B across 8 NeuronCores (2 examples per core);
W1/W2/vm replicated.  Softmax needs no max-subtraction: |scores| <= ||vm||_1
(~13 at this problem scale), safely inside fp32/bf16 exp range.

v3 layout (x cast to bf16 on host, plus a host-transposed copy xT; rel-err
budget 2e-2, measured ~2.5e-3):
  - 1024-row t-chunks; DMA granule = 2 chunks so every descriptor is 4 KiB.
  - xin [p,cc,r,m]: t-partitioned (t = c*1024 + p*8 + r), for pooling.
  - xts [p,mh,tau]: m-partitioned from xT, for the score path.
  - h^T = W1^T @ x^T on PE (bf16), split in 512-column halves to fit two
    double-buffered PSUM banks; tanh+bias on ACT per half.
  - scores: per 128-t block, stationary strided hsb slice (tau = q*8 + r)
    so score partitions line up with xin's t layout; rhs = vm column.
  - e = exp(s) -> e_all (bf16).
  - pooling on PE: acc_ps[m-half] += xin^T @ e_col, 1-column accumulating
    matmuls with x stationary (PSUM memset once; start=True would zero the
    whole bank row and wipe the sibling m-half chain).
  - software pipelining: scores/exp lag h by one chunk, pooling by two.
"""

import numpy as np
import ml_dtypes

import concourse.bass as bass
import concourse.tile as tile
from concourse import bacc, mybir
from concourse.bass_utils import run_bass_kernel_spmd

FP32 = mybir.dt.float32
BF16 = mybir.dt.bfloat16
AF = mybir.ActivationFunctionType

N_CORES = 8
B = 16
B_PER_CORE = B // N_CORES  # 2
T = 8192
M = 256
P = 128
CHUNK = 1024         # t rows per chunk
NT = CHUNK // P      # 8 t-rows per partition per chunk
NCHUNK = T // CHUNK  # 8 chunks per example
NE = NCHUNK * NT     # e columns per example (64)
GC = 2               # chunks per DMA granule
GRAN = GC * CHUNK    # 2048 t rows per granule
NGRAN = T // GRAN    # 4 granules per example
HALF = CHUNK // 2    # h-matmul column split (PSUM bank budget)


def _build_program() -> bass.Bass:
    nc = bacc.Bacc("TRN2", target_bir_lowering=False, debug=False)

    x = nc.dram_tensor("x", [B_PER_CORE, T, M], BF16, kind="ExternalInput")
    xT = nc.dram_tensor("xT", [B_PER_CORE, M, T], BF16, kind="ExternalInput")
    W1 = nc.dram_tensor("W1", [M, M], FP32, kind="ExternalInput")
    W2 = nc.dram_tensor("W2", [M, M], FP32, kind="ExternalInput")
    vm = nc.dram_tensor("vm", [1, M], FP32, kind="ExternalInput")
    out = nc.dram_tensor("out", [B_PER_CORE, M], FP32, kind="ExternalOutput")

    with tile.TileContext(nc) as tc:
        with (
            tc.tile_pool(name="setup", bufs=1) as setup,
            tc.tile_pool(name="xin", bufs=4) as xin_pool,
            tc.tile_pool(name="xts", bufs=4) as xts_pool,
            tc.tile_pool(name="hps", bufs=2, space="PSUM") as hps_pool,
            tc.tile_pool(name="hsb", bufs=2) as hsb_pool,
            tc.tile_pool(name="sps", bufs=2, space="PSUM") as sps_pool,
            tc.tile_pool(name="acc", bufs=1, space="PSUM") as acc_pool,
            tc.tile_pool(name="eee", bufs=1) as e_pool,
            tc.tile_pool(name="fin", bufs=2) as fin_pool,
        ):
            # ---------------- setup ----------------
            # W1 blocks: w1b[p, i, n] = W1[i*128+p, n], cast to bf16
            w1f = setup.tile([P, 2, M], FP32)
            nc.sync.dma_start(out=w1f, in_=W1.rearrange("(a p) n -> p a n", p=P))
            w1b = setup.tile([P, 2, M], BF16)
            nc.vector.tensor_copy(w1b, w1f)

            # W2 blocks (f32, setup only)
            w2f = setup.tile([P, 2, M], FP32)
            nc.sync.dma_start(out=w2f, in_=W2.rearrange("(a p) n -> p a n", p=P))

            # vm transposed: vmt[p, i] = vm[0, i*128+p]
            vmt_f = setup.tile([P, 2], FP32)
            nc.sync.dma_start(out=vmt_f, in_=vm[0].rearrange("(a p) -> p a", p=P))
            vmt_b = setup.tile([P, 2], BF16)
            nc.vector.tensor_copy(vmt_b, vmt_f)

            # c = vm @ W2, computed directly transposed: c_sb[p, nh] = c[nh*128+p]
            c_ps = sps_pool.tile([P, 2], FP32, tag="sps")
            for nh in range(2):
                for mh in range(2):
                    nc.tensor.matmul(
                        c_ps[:, nh : nh + 1],
                        lhsT=w2f[:, mh, nh * P : (nh + 1) * P],
                        rhs=vmt_f[:, mh : mh + 1],
                        start=(mh == 0),
                        stop=(mh == 1),
                    )
            c_sb = setup.tile([P, 2], FP32)
            nc.vector.tensor_copy(c_sb, c_ps)

            ones_col = setup.tile([P, 1], FP32)
            nc.vector.memset(ones_col, 1.0)
            ones_row = setup.tile([1, P], FP32)
            nc.vector.memset(ones_row, 1.0)

            # ---------------- main loop ----------------
            # One flat pipeline over B_PER_CORE * NCHUNK chunks with
            # granule-ahead DMA prefetch (also across the example boundary).
            NCT = B_PER_CORE * NCHUNK   # 16 global chunks
            NGT = B_PER_CORE * NGRAN    # 8 global granules

            xin_t = [None] * NCT
            xts_t = [None] * NCT
            hsb_t = [None] * NCT
            e_t = [None] * B_PER_CORE
            acc_t = [None] * B_PER_CORE

            def emit_dma(g, split=False):
                b, gb = divmod(g, NGRAN)
                # granule of 2 chunks; every descriptor 4 KiB contiguous.
                # xts first: the h-matmuls only need xts, xin is not read
                # until the pooling two chunks later.
                # split=True issues per-chunk transfers (first granule only)
                # so the pipeline can start on chunk 0 without waiting for
                # the whole granule.
                # xts[p, mh, tau] = x[b, gb*2048+tau, mh*128+p]
                if split:
                    for cc in range(GC):
                        xts_c = xts_pool.tile([P, 2, CHUNK], BF16, tag=f"xts0{cc}")
                        nc.sync.dma_start(
                            out=xts_c,
                            in_=xT[b, :, gb * GRAN + cc * CHUNK :
                                   gb * GRAN + (cc + 1) * CHUNK].rearrange(
                                "(a p) t -> p a t", p=P
                            ),
                        )
                        xts_t[g * GC + cc] = xts_c
                else:
                    xts = xts_pool.tile([P, 2, GRAN], BF16)
                    nc.sync.dma_start(
                        out=xts,
                        in_=xT[b, :, gb * GRAN : (gb + 1) * GRAN].rearrange(
                            "(a p) t -> p a t", p=P
                        ),
                    )
                    for cc in range(GC):
                        xts_t[g * GC + cc] = xts[:, :, cc * CHUNK : (cc + 1) * CHUNK]
                # xin[p, cc, r, m] = x[b, gb*2048 + cc*1024 + p*8 + r, m]
                xin = xin_pool.tile([P, GC, NT, M], BF16)
                nc.sync.dma_start(
                    out=xin,
                    in_=x[b, gb * GRAN : (gb + 1) * GRAN, :].rearrange(
                        "(cc p r) m -> p cc r m", p=P, r=NT
                    ),
                )
                for cc in range(GC):
                    xin_t[g * GC + cc] = xin[:, cc]

            def emit_h(ct):
                xts = xts_t[ct]
                hsb = hsb_pool.tile([P, 2, P, NT], BF16)
                for hf in range(2):
                    # h^T = W1^T @ x^T, 512-column half, acc over m-halves
                    hps = hps_pool.tile([P, 2, HALF], FP32)
                    for nh in range(2):
                        for mh in range(2):
                            nc.tensor.matmul(
                                hps[:, nh, :],
                                lhsT=w1b[:, mh, nh * P : (nh + 1) * P],
                                rhs=xts[:, mh, hf * HALF : (hf + 1) * HALF],
                                start=(mh == 0),
                                stop=(mh == 1),
                            )
                    # tanh with per-partition bias c into hsb[p, nh, q, r]
                    # over tau = q*8 + r (this half: q in [64*hf, 64*hf+64))
                    q0 = hf * (P // 2)
                    for nh in range(2):
                        nc.scalar.activation(
                            hsb[:, nh, q0 : q0 + P // 2, :],
                            hps[:, nh],
                            AF.Tanh,
                            bias=c_sb[:, nh : nh + 1],
                        )
                hsb_t[ct] = hsb

            def emit_scores(ct):
                # s[q, r] for t = c*1024 + q*8 + r: stationary strided hsb
                # slice [128 tau = q*8+r], moving vm column
                b, c = divmod(ct, NCHUNK)
                e_all = e_t[b]
                sps = sps_pool.tile([P, NT], FP32, tag="sps")
                hsb = hsb_t[ct]
                for r in range(NT):
                    for nh in range(2):
                        nc.tensor.matmul(
                            sps[:, r : r + 1],
                            lhsT=hsb[:, nh, :, r],
                            rhs=vmt_b[:, nh : nh + 1],
                            start=(nh == 0),
                            stop=(nh == 1),
                        )
                nc.scalar.activation(
                    e_all[:, c * NT : (c + 1) * NT],
                    sps,
                    AF.Exp,
                )
                hsb_t[ct] = None

            def emit_pool(ct):
                # acc_ps[q, mh] += sum_p x[t(p,r), mh*128+q] * e[t(p,r)]
                b, c = divmod(ct, NCHUNK)
                e_all = e_t[b]
                acc_ps = acc_t[b]
                xin = xin_t[ct]
                for r in range(NT):
                    for mh in range(2):
                        nc.tensor.matmul(
                            acc_ps[:, mh : mh + 1],
                            lhsT=xin[:, r, mh * P : (mh + 1) * P],
                            rhs=e_all[:, c * NT + r : c * NT + r + 1],
                            start=False,
                            stop=(c == NCHUNK - 1 and r == NT - 1),
                            skip_group_check=True,
                        )
                xin_t[ct] = None

            def emit_finalize(b):
                e_all = e_t[b]
                acc_ps = acc_t[b]
                # Z = sum(e_all): free-dim reduce on DVE, partition reduce on PE
                z_red = fin_pool.tile([P, 1], FP32)
                nc.vector.reduce_sum(z_red, e_all, axis=mybir.AxisListType.X)
                z_ps = sps_pool.tile([1, 1], FP32, tag="sps")
                nc.tensor.matmul(z_ps, lhsT=z_red, rhs=ones_col, start=True, stop=True)
                z_sb = fin_pool.tile([1, 1], FP32)
                nc.vector.tensor_copy(z_sb, z_ps)
                # broadcast Z to all partitions, then reciprocal
                zb_ps = sps_pool.tile([P, 1], FP32, tag="sps")
                nc.tensor.matmul(zb_ps, lhsT=ones_row, rhs=z_sb, start=True, stop=True)
                rz = fin_pool.tile([P, 1], FP32)
                nc.vector.reciprocal(rz, zb_ps)
                # scale pooled sums by 1/Z; acc_ps is already m-partitioned
                outsb = fin_pool.tile([P, 2], FP32)
                nc.vector.tensor_scalar_mul(outsb, acc_ps, rz)
                nc.sync.dma_start(
                    out=out[b].rearrange("(a p) -> p a", p=P), in_=outsb
                )

            for b in range(B_PER_CORE):
                e_t[b] = e_pool.tile([P, NE], BF16, name=f"e_all{b}")
                # start=True zeroes the whole PSUM bank row, so the two
                # m-half accumulation chains sharing this tile would wipe
                # each other; memset once and accumulate-only instead.
                acc_t[b] = acc_pool.tile([P, 2], FP32, name=f"acc{b}")
                nc.vector.memset(acc_t[b], 0.0)

            emit_dma(0, split=True)
            for g in range(1, 3):
                emit_dma(g)
            for ct in range(NCT):
                if ct % GC == 0 and ct // GC + 3 < NGT:
                    emit_dma(ct // GC + 3)
                emit_h(ct)
                if ct >= 1:
                    emit_scores(ct - 1)
                if ct >= 2:
                    emit_pool(ct - 2)
                if ct == NCHUNK + 1:
                    emit_finalize(0)
            emit_scores(NCT - 1)
            emit_pool(NCT - 2)
            emit_pool(NCT - 1)
            emit_finalize(1)

    return nc


_PROGRAM_CACHE: list = []


def _get_program() -> bass.Bass:
    if not _PROGRAM_CACHE:
        nc = _build_program()
        nc.finalize()
        _PROGRAM_CACHE.append(nc)
    return _PROGRAM_CACHE[0]


def _make_in_maps(x, W1, W2, vm):
    xb = np.ascontiguousarray(x).astype(ml_dtypes.bfloat16)
    xbT = np.ascontiguousarray(xb.transpose(0, 2, 1))
    W1 = np.ascontiguousarray(W1, dtype=np.float32)
    W2 = np.ascontiguousarray(W2, dtype=np.float32)
    vm = np.ascontiguousarray(vm, dtype=np.float32)
    return [
        {
            "x": xb[i * B_PER_CORE : (i + 1) * B_PER_CORE],
            "xT": xbT[i * B_PER_CORE : (i + 1) * B_PER_CORE],
            "W1": W1,
            "W2": W2,
            "vm": vm,
        }
        for i in range(N_CORES)
    ]


def kernel(x, W1, W2, vm):
    nc = _get_program()
    in_maps = _make_in_maps(x, W1, W2, vm)
    res = run_bass_kernel_spmd(nc, in_maps, list(range(N_CORES)))
    out = np.concatenate([res.results[i]["out"] for i in range(N_CORES)], axis=0)
    return out.reshape(B, 1, M).astype(np.float32)
